# revision 52
# baseline (speedup 1.0000x reference)
"""Dilated local attention (3x3 window, dilation 2) on 8 trn2 NeuronCores.

Problem: B=8, DIM=256, H=W=64, N=4096.
  k_u = unfold(k, 3x3, dil=2, pad=2)            [B, 256, 9, N]   (zero pad)
  attn = softmax(einsum(bdn,bdkn->bkn)/16, k)   [B, 9, N]
  out  = einsum(bkn,bdkn->bdn)                  [B, 256, N]

Sharding: pure data parallel, one batch element per core.

Per-core design (fp16 on chip, channels on partitions, 2 chunks of 128):
  p1  products pr_k = q * shift_k(kp) on DVE/GPSIMD (fp16, 2x mode), then a
      TRANSPOSED one-hot reduce on PE: lhsT = pr slice over a row-pair of the
      image (128 pixels), rhs = one-hot column block (value 1/16), giving
      scores directly in pixel-on-partition layout s_T[128px, 9] in PSUM.
      The channel reduction is nearly free on PE (free-size 9 per matmul)
      and score PSUM shrinks to one bank.
  smx softmax entirely in the transposed layout: ACT exp -> e_T fp16; DVE
      free-dim reduce -> den; DVE reciprocal; DVE multiply -> attn_T fp16.
      Padded taps have score exactly 0 (zero-padded kp) so exp(0)=1 enters
      the denominator, matching the reference's softmax over padded logits.
  tr  PE transpose per 128px block -> attn rows [9, 512] PSUM -> ACT evac.
  bc  two-stage DMA broadcast per quarter: rows -> 8 copies [72, 1024] ->
      stride-0 re-read to bc[128, 1024] fp16 (no PE broadcast, no big ACT
      evacuations); issues split across the SP and ACT queues.
  p2  quarter products v_shift * bc on DVE/GPSIMD (chunk-pair via stride-0
      bc view), accumulated over the 9 offsets by PE identity matmuls into
      PSUM out tiles, ACT-evacuated to fp16 and DMA'd out (no add tree).

PSUM (8 banks): "st" 1 bank (score tiles, slice-sequential accumulation
groups), "at" 1 bank (attn transpose-back), OA/OB/OC 2 banks each (out
accumulators; a quarter's two slices use two tags, rotation mod 3 so each
new out tile only waits on an evac two slices back).

Engines: DVE and GPSIMD split the product work (~88us of DVE-equivalent at
a 1:3.65 rate ratio); PE does all reductions/transposes/accumulations; ACT
does exp + evacuations; DMA does loads/broadcasts/stores. Inputs are
loaded as per-half tiles (kp/vp with halo rows) so full-chunk products get
precise, early-resolving DMA dependencies. Phases are emitted sequentially
(p1h0, p1h1, then p2 per quarter) - the overlap emerges from cross-engine
pipelining, not interleaved emission. TimelineSim: ~86.4us/core (baseline
was 135.4us).
"""

import numpy as np

B, DIM, H, W = 8, 256, 64, 64
N = H * W
KS, DIL, PAD = 3, 2, 2
HP, WP = H + 2 * PAD, W + 2 * PAD  # 68, 68
NP = HP * WP  # 4624
NCHUNK = 2
P = 128
NCORES = 8
HH = H // 2      # image rows per half (32)
NH = HH * W      # pixels per half (2048)
SL = 512         # pixels per slice (8 image rows); 8 slices
QW = 1024        # pixels per quarter (16 image rows); 4 quarters

_CACHE = {}

# products executed on gpsimd instead of DVE (load balancing)
POOL_P1 = {(0, 4), (1, 4), (1, 8)}   # (half, k)
POOL_P2 = {
    (0, 1), (0, 4), (0, 7),
    (1, 1), (1, 4),
    (2, 1), (2, 4),
    (3, 1),
}  # (quarter, j): early js, light in the final quarter
SPLIT_P1 = {(0, 0), (0, 1), (0, 2)}  # chunk-split for early DMA overlap
OUT_TAGS = ("OA", "OB", "OC")


def _build_program():
    import concourse.bacc as bacc
    import concourse.tile as tile
    import concourse.mybir as mybir

    f16 = mybir.dt.float16
    f32 = mybir.dt.float32
    MULT = mybir.AluOpType.mult
    ADD = mybir.AluOpType.add
    AF = mybir.ActivationFunctionType
    AX = mybir.AxisListType

    nc = bacc.Bacc("TRN2", target_bir_lowering=False, debug=False)

    q_d = nc.dram_tensor("q", [P, NCHUNK, N], f16, kind="ExternalInput").ap()
    kp_d = nc.dram_tensor("kp", [P, NCHUNK, NP], f16, kind="ExternalInput").ap()
    vp_d = nc.dram_tensor("vp", [P, NCHUNK, NP], f16, kind="ExternalInput").ap()
    oh_d = nc.dram_tensor("oh", [P, 81], f16, kind="ExternalInput").ap()
    eye_d = nc.dram_tensor("eye", [P, P], f16, kind="ExternalInput").ap()
    out_d = nc.dram_tensor("out", [P, NCHUNK, N], f16, kind="ExternalOutput").ap()

    offs = [(di * DIL, dj * DIL) for di in range(-1, 2) for dj in range(-1, 2)]

    with tile.TileContext(nc) as tc:
        with (
            tc.tile_pool(name="inp", bufs=1) as inp,
            tc.tile_pool(name="kpp", bufs=1) as kpp,
            tc.tile_pool(name="cst", bufs=1) as cst,
            tc.tile_pool(name="prp", bufs=10) as prp,
            tc.tile_pool(name="p2p", bufs=5) as p2p,
            tc.tile_pool(name="att", bufs=1) as att,
            tc.tile_pool(name="bcp", bufs=17) as bcp,
            tc.tile_pool(name="outp", bufs=2) as outp,
            tc.tile_pool(name="ps", bufs=1, space="PSUM") as psp,
        ):
            # per-half input tiles: a full-chunk product then depends on
            # exactly one tile's DMA pieces (precise, early-resolving deps).
            # kp/vp halves carry halo rows: h0 = padded rows 0:36,
            # h1 = padded rows 30:68 (6 rows double-loaded).
            KR = (36, 38)            # kp/vp rows per half-tile
            KLO = (0, 30)            # first padded row of each half-tile
            q_sbs = [inp.tile([P, NCHUNK, NH], f16, tag=f"q{h}", name=f"q{h}")
                     for h in range(2)]
            vp_sbs = [inp.tile([P, NCHUNK, KR[h] * WP], f16, tag=f"vp{h}",
                               name=f"vp{h}") for h in range(2)]
            kp_sbs = [kpp.tile([P, NCHUNK, KR[h] * WP], f16, tag=f"kp{h}",
                               name=f"kp{h}") for h in range(2)]
            oh_sb = cst.tile([P, 81], f16, tag="oh")
            eye_sb = cst.tile([P, P], f16, tag="eye")

            first = True
            for h in range(2):
                lo, hi = KLO[h] * WP, (KLO[h] + KR[h]) * WP
                mid, midp = NH // 2, (hi - lo) // 2
                for c in range(NCHUNK):
                    nc.scalar.dma_start(
                        q_sbs[h][:, c, :mid], q_d[:, c, h * NH : h * NH + mid]
                    )
                    nc.scalar.dma_start(
                        kp_sbs[h][:, c, :midp], kp_d[:, c, lo : lo + midp]
                    )
                    nc.scalar.dma_start(
                        q_sbs[h][:, c, mid:], q_d[:, c, h * NH + mid : (h + 1) * NH]
                    )
                    nc.scalar.dma_start(
                        kp_sbs[h][:, c, midp:], kp_d[:, c, lo + midp : hi]
                    )
                    if first:
                        nc.scalar.dma_start(oh_sb[:, :], oh_d)
                        first = False
            nc.scalar.dma_start(eye_sb[:, :], eye_d)
            for h in range(2):
                lo, hi = KLO[h] * WP, (KLO[h] + KR[h]) * WP
                for c in range(NCHUNK):
                    nc.scalar.dma_start(vp_sbs[h][:, c, :], vp_d[:, c, lo:hi])

            q_vs = [q_sbs[h][:, :, :].rearrange("p c (r w) -> p c r w", r=HH)
                    for h in range(2)]
            kp_vs = [kp_sbs[h][:, :, :].rearrange("p c (r w) -> p c r w", r=KR[h])
                     for h in range(2)]
            vp_vs = [vp_sbs[h][:, :, :].rearrange("p c (r w) -> p c r w", r=KR[h])
                     for h in range(2)]

            # pre-warm the ACT Exp table during input DMA
            warm = att.tile([1, 8], f32, tag="warm")
            nc.vector.memset(warm[:, :], 1.0)
            nc.scalar.activation(warm[:, :], warm[:, :], AF.Exp)

            # per-half transposed-softmax tiles
            eT = [att.tile([P, 16, 9], f16, tag=f"eT{h}", name=f"eT{h}") for h in range(2)]
            rec = [att.tile([P, 16], f32, tag=f"rec{h}", name=f"rec{h}") for h in range(2)]
            aT = [att.tile([P, 16, 9], f16, tag=f"aT{h}", name=f"aT{h}") for h in range(2)]
            a_sb = [att.tile([9, NH], f16, tag=f"a{h}", name=f"arow{h}") for h in range(2)]

            sT = {}      # slice -> psum score tile [P, 4, 9]
            outp_ps = {}  # slice -> psum out tile [P, 2, SL]

            def p1_product(h, k):
                """pr = q(half) * kp shifted by offs[k]; fp16 [P, 2, NH].
                Chunk-split products start as soon as one chunk is loaded."""
                di, dj = offs[k]
                rk = PAD + di + h * HH - KLO[h]  # row within the half kp tile
                pr = prp.tile([P, NCHUNK, NH], f16, tag="pr")
                pr_v = pr[:, :, :].rearrange("p c (r w) -> p c r w", r=HH)
                eng = nc.gpsimd if (h, k) in POOL_P1 else nc.vector
                # gpsimd products also chunk-split (nearly free on Q7) so
                # the Pool engine starts as soon as chunk-0 DMA lands
                csplit = (
                    [(c, c + 1) for c in range(NCHUNK)]
                    if (h, k) in SPLIT_P1 or (h, k) in POOL_P1
                    else [(0, NCHUNK)]
                )
                for c0, c1 in csplit:
                    eng.tensor_tensor(
                        pr_v[:, c0:c1],
                        q_vs[h][:, c0:c1, :, :],
                        kp_vs[h][
                            :, c0:c1, rk : rk + HH, PAD + dj : PAD + dj + W
                        ],
                        MULT,
                    )
                return pr

            def p1_reduce(h, prods):
                """transposed one-hot matmuls, slice-sequential so the single
                "st" PSUM bank has one pending accumulation group at a time.
                All four nests are emitted before the softmaxes so the PE
                nest chain (which releases the pr buffers) is never blocked
                behind a softmax round-trip."""
                for sl in range(4):
                    S = 4 * h + sl
                    sT[S] = psp.tile([P, 4, 9], f32, tag="st", name=f"sT{S}")
                    for blk in range(4):
                        r = 8 * sl + 2 * blk  # row-pair within half
                        for k in range(9):
                            pr_v = prods[k][:, :, :].rearrange(
                                "p c (r w) -> p c r w", r=HH
                            )
                            for c in range(NCHUNK):
                                nc.tensor.matmul(
                                    sT[S][:, blk, :],
                                    pr_v[:, c, r : r + 2, :],
                                    oh_sb[:, 9 * k : 9 * k + 9],
                                    start=(k == 0 and c == 0),
                                    stop=(k == 8 and c == 1),
                                )
                for sl in range(4):
                    smax_slice(4 * h + sl)
                    if sl == 1:
                        smax_quarter(2 * h)
                        bcs_by_q[2 * h] = bc_quarter(2 * h)
                    elif sl == 3:
                        smax_quarter(2 * h + 1)
                        bcs_by_q[2 * h + 1] = bc_quarter(2 * h + 1)

            def smax_slice(S):
                """exp in pixel-on-partition layout (per slice; reads the
                slice's score PSUM tile)."""
                h, sl = divmod(S, 4)
                eT_s = eT[h][:, 4 * sl : 4 * sl + 4, :]
                nc.scalar.activation(eT_s, sT[S][:, :, :], AF.Exp)

            def smax_quarter(q):
                """den/recip/normalize batched over the quarter's 2 slices,
                then per-slice transpose-back + evac."""
                h, qh = divmod(q, 2)
                lo = 8 * qh
                eT_q = eT[h][:, lo : lo + 8, :]
                den = rec[h][:, lo : lo + 8].unsqueeze(2)
                nc.vector.tensor_reduce(den, eT_q, AX.X, ADD)
                nc.vector.reciprocal(den, den)
                aT_q = aT[h][:, lo : lo + 8, :]
                nc.vector.tensor_tensor(
                    aT_q, eT_q, den.broadcast_to([P, 8, 9]), MULT
                )
                for sl in (2 * qh, 2 * qh + 1):
                    S = 4 * h + sl
                    at_ps = psp.tile([9, SL], f16, tag="at", name=f"at{S}")
                    for blk in range(4):
                        nc.tensor.transpose(
                            at_ps[:, P * blk : P * (blk + 1)],
                            aT[h][:, 4 * sl + blk, :],
                            eye_sb[:, :],
                        )
                    nc.scalar.activation(
                        a_sb[h][:, SL * sl : SL * (sl + 1)], at_ps[:, :], AF.Copy
                    )

            def bc_quarter(q):
                """two-stage DMA broadcast of the 9 attn rows of quarter q.
                Issues split across the SP and ACT queues (q1 on ACT) so
                neither sequencer becomes the head-of-line bottleneck."""
                h, qh = divmod(q, 2)
                dma_eng = nc.scalar if q == 1 else nc.sync
                rows = a_sb[h][:, QW * qh : QW * (qh + 1)]
                a8 = att.tile([72, QW], f16, tag=f"a8_{q % 2}", name=f"a8_{q}")
                dma_eng.dma_start(
                    a8[:, :], rows.unsqueeze(1).broadcast_to([9, 8, QW])
                )
                bcs = {}
                for k in range(9):
                    bc = bcp.tile([P, QW], f16, tag="bc")
                    dma_eng.dma_start(
                        bc[:, :],
                        a8[8 * k : 8 * k + 8, :].unsqueeze(1).broadcast_to(
                            [8, 16, QW]
                        ),
                    )
                    bcs[k] = bc
                return bcs

            def p2_product(q, j, bc):
                """p2 = v_shift * bc (quarter; both chunks via stride-0 bc)."""
                di, dj = offs[j]
                h = q // 2
                rk = PAD + di + 16 * q - KLO[h]  # row within the half vp tile
                p2 = p2p.tile([P, NCHUNK, QW], f16, tag="p2")
                p2_v = p2[:, :, :].rearrange("p c (r w) -> p c r w", r=16)
                bc_v = (
                    bc[:, :]
                    .rearrange("p (r w) -> p r w", r=16)
                    .unsqueeze(1)
                    .broadcast_to([P, NCHUNK, 16, W])
                )
                eng = nc.gpsimd if (q, j) in POOL_P2 else nc.vector
                eng.tensor_tensor(
                    p2_v,
                    vp_vs[h][
                        :, :, rk : rk + 16, PAD + dj : PAD + dj + W
                    ],
                    bc_v,
                    MULT,
                )
                return p2

            def p2_accum(q, j, p2):
                """identity matmuls accumulate p2 into the 2 slices' psum."""
                for s2 in range(2):
                    S = 2 * q + s2
                    if j == 0:
                        outp_ps[S] = psp.tile(
                            [P, NCHUNK, SL], f32, tag=OUT_TAGS[S % 3], name=f"o{S}"
                        )
                    for c in range(NCHUNK):
                        nc.tensor.matmul(
                            outp_ps[S][:, c, :],
                            eye_sb[:, :],
                            p2[:, c, SL * s2 : SL * (s2 + 1)],
                            start=(j == 0),
                            stop=(j == 8),
                        )

            def out_slice(S):
                # early slices' DMAs issue from ACT (SP is blocked waiting on
                # h1 broadcast deps then); late slices' from the freed SP.
                ob = outp.tile([P, NCHUNK, SL], f16, tag="ob")
                if S == 7:
                    # last slice: per-chunk evacs on DVE and ACT in parallel,
                    # DMAs issued from SP while ACT issues S6's
                    for c in range(NCHUNK):
                        if c == 0:
                            nc.vector.tensor_copy(
                                ob[:, c, :], outp_ps[S][:, c, :]
                            )
                        else:
                            nc.scalar.activation(
                                ob[:, c, :], outp_ps[S][:, c, :], AF.Copy
                            )
                        nc.sync.dma_start(
                            out_d[:, c, SL * S : SL * (S + 1)], ob[:, c, :]
                        )
                else:
                    dma_eng = nc.scalar if S < 4 or S == 6 else nc.sync
                    nc.scalar.activation(ob[:, :, :], outp_ps[S][:, :, :], AF.Copy)
                    dma_eng.dma_start(
                        out_d[:, :, SL * S : SL * (S + 1)], ob[:, :, :]
                    )

            def p1_phase(h):
                prods = {}
                for hk, k in sorted(POOL_P1):
                    if hk == h:
                        prods[k] = p1_product(h, k)
                # split ks: pieces emitted c-major (and row-quarter pieces
                # for k0/k1) so DVE starts on the very first DMA arrivals
                split = sorted(k for hk, k in SPLIT_P1 if hk == h)
                for k in split:
                    if k not in prods:
                        prods[k] = prp.tile(
                            [P, NCHUNK, NH], f16, tag="pr", name=f"pr{h}_{k}"
                        )
                pieces = []
                for c in range(NCHUNK):
                    for rh in (0, 1):
                        for k in split:
                            if k < 2 and h == 0:
                                pieces.append((k, c, rh))
                            elif rh == 0:
                                pieces.append((k, c, None))
                for k, c, rh in pieces:
                    di, dj = offs[k]
                    rk = PAD + di + h * HH - KLO[h]
                    pr_v = prods[k][:, :, :].rearrange(
                        "p c (r w) -> p c r w", r=HH
                    )
                    r0, r1 = (0, HH) if rh is None else (16 * rh, 16 * rh + 16)
                    nc.vector.tensor_tensor(
                        pr_v[:, c : c + 1, r0:r1],
                        q_vs[h][:, c : c + 1, r0:r1, :],
                        kp_vs[h][
                            :,
                            c : c + 1,
                            rk + r0 : rk + r1,
                            PAD + dj : PAD + dj + W,
                        ],
                        MULT,
                    )
                for k in range(9):
                    if k not in prods:
                        prods[k] = p1_product(h, k)
                p1_reduce(h, prods)

            # ---------------- emission schedule ----------------
            bcs_by_q = {}
            p1_phase(0)
            p1_phase(1)
            for q in range(4):
                bcs = bcs_by_q[q]
                prods = {}
                for qq, j in sorted(POOL_P2):
                    if qq == q:
                        prods[j] = p2_product(q, j, bcs[j])
                for j in range(9):
                    if q == 3 and j == 8:
                        # last product of the program: chunk-split with
                        # interleaved accums so the final evac starts sooner
                        di, dj = offs[j]
                        rk = PAD + di + 48 - KLO[1]
                        p2 = p2p.tile([P, NCHUNK, QW], f16, tag="p2", name="p2last")
                        p2_v = p2[:, :, :].rearrange("p c (r w) -> p c r w", r=16)
                        bc_v3 = bcs[j][:, :].rearrange("p (r w) -> p r w", r=16)
                        for c in range(NCHUNK):
                            nc.vector.tensor_tensor(
                                p2_v[:, c],
                                vp_vs[1][:, c, rk : rk + 16, PAD + dj : PAD + dj + W],
                                bc_v3,
                                MULT,
                            )
                            for s2 in range(2):
                                S = 2 * q + s2
                                nc.tensor.matmul(
                                    outp_ps[S][:, c, :],
                                    eye_sb[:, :],
                                    p2[:, c, SL * s2 : SL * (s2 + 1)],
                                    start=False,
                                    stop=True,
                                )
                        continue
                    if j not in prods:
                        prods[j] = p2_product(q, j, bcs[j])
                    p2_accum(q, j, prods[j])
                for s2 in range(2):
                    out_slice(2 * q + s2)

    nc.compile()
    return nc


def _host_inputs(q, k, v):
    """q,k,v: [B, DIM, N] float32 -> list of per-core input dicts."""
    qh = q.astype(np.float16).reshape(B, NCHUNK, P, N).transpose(0, 2, 1, 3)
    ki = k.astype(np.float16).reshape(B, DIM, H, W)
    vi = v.astype(np.float16).reshape(B, DIM, H, W)
    kp = np.zeros((B, DIM, HP, WP), np.float16)
    vp = np.zeros((B, DIM, HP, WP), np.float16)
    kp[:, :, PAD : PAD + H, PAD : PAD + W] = ki
    vp[:, :, PAD : PAD + H, PAD : PAD + W] = vi
    kp = kp.reshape(B, NCHUNK, P, NP).transpose(0, 2, 1, 3)
    vp = vp.reshape(B, NCHUNK, P, NP).transpose(0, 2, 1, 3)

    oh = np.zeros((P, 81), np.float16)
    for k9 in range(9):
        oh[:, 9 * k9 + k9] = 1.0 / 16.0
    eye = np.eye(P, dtype=np.float16)

    ins = []
    for b in range(B):
        ins.append(
            {
                "q": np.ascontiguousarray(qh[b]),
                "kp": np.ascontiguousarray(kp[b]),
                "vp": np.ascontiguousarray(vp[b]),
                "oh": oh,
                "eye": eye,
            }
        )
    return ins


def kernel(q, k, v, h=H, w=W, _trace=False):
    from concourse.bass_utils import run_bass_kernel_spmd

    q = np.asarray(q, np.float32)
    k = np.asarray(k, np.float32)
    v = np.asarray(v, np.float32)

    if "nc" not in _CACHE:
        _CACHE["nc"] = _build_program()
    nc = _CACHE["nc"]

    ins = _host_inputs(q, k, v)
    res = run_bass_kernel_spmd(nc, ins, core_ids=list(range(NCORES)), trace=_trace)

    outs = []
    for b in range(B):
        o = res.results[b]["out"]  # [128, 2, 4096] fp16
        outs.append(o.transpose(1, 0, 2).reshape(DIM, N))
    full = np.stack(outs).astype(np.float32)
    if _trace:
        return full, res
    return full
